# revision 2
# baseline (speedup 1.0000x reference)
# kernel_fused.py — CrystalGCNEncoder fused single-launch kernel for 8 trn2 cores.
#
# The baseline ran 7 SPMD launches with the full per-layer feature table
# replicated to every core over the (slow, ~45-60 MB/s) axon tunnel: ~600 MB
# of H2D per timed call -> ~32 s.  This version runs the WHOLE model (3 GAT
# layers + global-attention pooling) in ONE launch, and packs ALL per-core
# inputs into a single [128, MCOLS] bf16 array (int16 / f32 regions are
# bitcast) because each separate jax input array costs ~10 ms of tunnel
# overhead:
#   - per-core payload: node-feature shard, edge src-slot (int16) and
#     dst-position columns, graph ids, and a 1/8 column-shard of every
#     weight matrix (weights are AllGathered on device; the tunnel ships
#     per-core copies of replicated arrays, so replication is pure waste)
#   - cross-core feature exchange per layer is an on-device bf16 AllGather
#     of [nloc, Dout+4] -> [nstar, Dout+4] (feat rows carry the el attention
#     half in a 4-col tail; er stays core-local)
#   - one-hot scatter/gather matrices, iota/identity constants, ones rows
#     and graph one-hots are built on device (is_equal against iota)
#   - per-edge source rows are fetched with per-chunk indirect DMA gathers
#     (the HW DGE reads ONE offset per partition and copies that partition's
#     whole output row contiguously, so each gather moves one 128-edge chunk)
#   - the PJRT callable is jitted once and cached; per call we only pay the
#     single-array H2D + execution + single-array D2H
import numpy as np
import ml_dtypes

N, E, G = 20000, 320000, 200
F_IN, HID, H, LAT = 128, 128, 4, 128
O1, O2, O3 = HID // 2, HID, 2 * HID
D1, D2, D3 = H * O1, H * O2, H * O3          # 256, 512, 1024
NEG_SLOPE = 0.2
NCORES = 8
BF16 = ml_dtypes.bfloat16

NTILES = 21
CPT = 16                 # edge chunks (of 128) per node tile
TPE = CPT * 128          # edges per tile
NLOC = NTILES * 128      # node slots per core
NSTAR = NLOC * NCORES
NCH = NTILES * CPT       # edge chunks per core
GPC = G // NCORES
GPAD = 32
DOUTS = [D1, D2, D3]
KS = [F_IN // 128, D1 // 128, D2 // 128]
DCS = [(d + 8) // 8 for d in DOUTS]          # weight col-shard width per core

# mega input column layout (bf16 columns); node features ride as fp8-e4m3
# (bitcast, 2 per bf16 col) — tested rel err 7.4e-3 vs 5.0e-3 all-bf16
C_X1T = 0                                    # NLOC/2 bf16 cols of fp8 x1T
C_DPOS = C_X1T + NLOC // 2                   # 1344  (int8 bitcast, NCH/2 cols)
C_GID = C_DPOS + NCH // 2                    # +168
C_IDX = C_GID + NTILES                       # +21   (int16 bitcast)
C_W = C_IDX + NCH                            # +336
WCOLS = sum(KS[l] * DCS[l] for l in range(3)) + 3 * 128   # 679 + 384
WOFFS = [0, KS[0] * DCS[0], KS[0] * DCS[0] + KS[1] * DCS[1]]
WPOFF = WOFFS[2] + KS[2] * DCS[2]            # pool weights offset inside W
C_SM = C_W + WCOLS                           # f32 bitcast region (19 f32)
NSM = 19
MCOLS = C_SM + 2 * NSM
# smalls f32 columns: 0 bg1c | 1 Wg2 | 2 bg2r | 3..16 bias chunks | 17 bmu | 18 blv
BOFF128 = [0, D1 // 128, (D1 + D2) // 128]


def _colchunks(c):
    out, s = [], 0
    while s < c:
        w = min(512, c - s)
        out.append((s, w))
        s += w
    return out


# ------------------------------------------------------------------ host prep
def host_prep(node_feat, src, dst, graph_ids):
    node_feat = np.asarray(node_feat, np.float32)
    src = np.asarray(src).astype(np.int64)
    dst = np.asarray(dst).astype(np.int64)
    graph_ids = np.asarray(graph_ids).astype(np.int64)

    gbounds = np.arange(NCORES + 1) * GPC
    nbounds = np.searchsorted(graph_ids, gbounds)
    core_of_node = np.searchsorted(nbounds, np.arange(N), side="right") - 1
    indeg = np.bincount(dst, minlength=N)

    glob2slot = np.zeros(N, np.int64)
    tile_of_node = np.zeros(N, np.int64)
    slotpos_of_node = np.zeros(N, np.int64)
    for c in range(NCORES):
        nodes = np.arange(nbounds[c], nbounds[c + 1])
        assert len(nodes) <= NLOC
        order = nodes[np.argsort(-indeg[nodes], kind="stable")]
        loads = np.zeros(NTILES, np.int64)
        counts = np.zeros(NTILES, np.int64)
        for nd in order:
            free = np.nonzero(counts < 128)[0]
            tgt = free[np.argmin(loads[free])]
            tile_of_node[nd] = tgt
            slotpos_of_node[nd] = counts[tgt]
            glob2slot[nd] = c * NLOC + tgt * 128 + counts[tgt]
            counts[tgt] += 1
            loads[tgt] += indeg[nd]
        assert loads.max() <= TPE

    edge_core = core_of_node[dst]
    idx_l, dpos_l, gid_l = [], [], []
    for c in range(NCORES):
        eids = np.nonzero(edge_core == c)[0]
        src_slot = np.zeros(NTILES * TPE, np.int64)
        dst_pos = np.full(NTILES * TPE, -1, np.int64)
        et = tile_of_node[dst[eids]]
        for t in range(NTILES):
            sel = eids[et == t]
            assert len(sel) <= TPE
            b = t * TPE
            src_slot[b : b + len(sel)] = glob2slot[src[sel]]
            dst_pos[b : b + len(sel)] = slotpos_of_node[dst[sel]]
        idx_l.append(np.ascontiguousarray(
            src_slot.reshape(NCH, 128).T).astype(np.int16))
        dpos_l.append(np.ascontiguousarray(
            dst_pos.reshape(NCH, 128).T).astype(np.int8))
        gid = np.full((NTILES, 128), -1, np.int64)
        nodes = np.arange(nbounds[c], nbounds[c + 1])
        gid[tile_of_node[nodes], slotpos_of_node[nodes]] = \
            graph_ids[nodes] - c * GPC
        gid_l.append(np.ascontiguousarray(gid.T).astype(BF16))

    x1 = np.zeros((NSTAR, F_IN), np.float32)
    x1[glob2slot] = node_feat
    x1T_l = [np.ascontiguousarray(x1[c * NLOC:(c + 1) * NLOC].T).astype(
        ml_dtypes.float8_e4m3) for c in range(NCORES)]
    return dict(idx=idx_l, dpos=dpos_l, gid=gid_l, x1T=x1T_l)


def fold_weights(W, al, ar):
    Din, D = W.shape
    Hh, O = al.shape
    Wl = np.einsum("iho,ho->ih", W.reshape(Din, Hh, O), al)
    Wr = np.einsum("iho,ho->ih", W.reshape(Din, Hh, O), ar)
    return np.concatenate([W, Wl, Wr], 1).astype(np.float32)


def wstack(Waug):
    """[Din, C] -> [128, Din/128, C] (partition-major K chunks)."""
    Din, C = Waug.shape
    return np.ascontiguousarray(Waug.reshape(Din // 128, 128, C).transpose(1, 0, 2))


# ------------------------------------------------------------------ builder
def build_fused(debug=False):
    import concourse.bass as bass
    import concourse.tile as tile
    from concourse import bacc, mybir

    bf = mybir.dt.bfloat16
    f32 = mybir.dt.float32
    f8 = mybir.dt.float8e4
    i16 = mybir.dt.int16
    i32 = mybir.dt.int32
    FT = mybir.ActivationFunctionType
    AO = mybir.AluOpType

    nc = bacc.Bacc("TRN2", target_bir_lowering=False, debug=False,
                   num_devices=NCORES)
    mega = nc.dram_tensor("mega", [128, MCOLS], bf, kind="ExternalInput").ap()
    mulv = nc.dram_tensor("mulv", [GPAD, 256], bf, kind="ExternalOutput").ap()
    if debug:
        dbgA = nc.dram_tensor("dbgA", [NSTAR, D1 + 4], bf,
                              kind="ExternalOutput").ap()
        dbgB = nc.dram_tensor("dbgB", [128, 2 * NLOC], bf,
                              kind="ExternalOutput").ap()
        dbgC = nc.dram_tensor("dbgC", [128, 4 * NTILES], bf,
                              kind="ExternalOutput").ap()

    with tile.TileContext(nc) as tc:
        with tc.tile_pool(name="cp", bufs=1) as cp, \
             tc.tile_pool(name="dram", bufs=1, space="DRAM") as dp:
            # ---------- persistent SBUF state from the packed input
            idx16 = cp.tile([128, NCH], i16)
            nc.sync.dma_start(idx16[:],
                              mega[:, C_IDX:C_IDX + NCH].bitcast(i16))
            idxs = cp.tile([128, NCH], i32)
            nc.vector.tensor_copy(idxs[:], idx16[:])
            dpos8 = cp.tile([128, NCH], mybir.dt.int8)
            nc.sync.dma_start(dpos8[:],
                              mega[:, C_DPOS:C_DPOS + NCH // 2].bitcast(
                                  mybir.dt.int8))
            dposs = cp.tile([128, NCH], bf)
            nc.vector.tensor_copy(dposs[:], dpos8[:])
            gids = cp.tile([128, NTILES], bf)
            nc.sync.dma_start(gids[:], mega[:, C_GID:C_GID + NTILES])
            smalls = cp.tile([128, NSM], f32)
            nc.sync.dma_start(smalls[:],
                              mega[:, C_SM:C_SM + 2 * NSM].bitcast(f32))
            # device-built constants
            io32 = cp.tile([128, 128], i32)
            nc.gpsimd.iota(io32[:], pattern=[[1, 128]], base=0,
                           channel_multiplier=0)
            ro32 = cp.tile([128, 128], i32)
            nc.gpsimd.iota(ro32[:], pattern=[[0, 128]], base=0,
                           channel_multiplier=1)
            iotas = cp.tile([128, 128], bf)
            nc.vector.tensor_copy(iotas[:], io32[:])
            rows = cp.tile([128, 128], bf)
            nc.vector.tensor_copy(rows[:], ro32[:])
            identb = cp.tile([128, 128], bf)
            nc.vector.tensor_tensor(out=identb[:], in0=rows[:], in1=iotas[:],
                                    op=AO.is_equal)
            idf32 = cp.tile([128, 128], f32)
            nc.vector.tensor_tensor(out=idf32[:], in0=rows[:], in1=iotas[:],
                                    op=AO.is_equal)
            on1f = cp.tile([1, 128], f32)
            nc.vector.memset(on1f[:], 1.0)
            # bias rows: transpose each of smalls cols 3..18 into a [1, 128]
            # row at partition 0 (matmul rhs must be partition-0 based)
            bros = cp.tile([1, 16 * 128], f32)
            Wg2c = cp.tile([128, 1], bf)
            nc.vector.tensor_copy(Wg2c[:], smalls[:, 1:2])
            # weight shards -> one DRAM bounce -> AllGather -> SBUF
            win = dp.tile([128, WCOLS], bf)
            nc.gpsimd.dma_start(win[:], mega[:, C_W:C_W + WCOLS])
            wout = dp.tile([NCORES, 128, WCOLS], bf, addr_space="Shared")
            nc.gpsimd.collective_compute(
                "AllGather", mybir.AluOpType.bypass,
                ins=[win.opt()], outs=[wout.opt()],
                replica_groups=[list(range(NCORES))])
            Waus = []
            for l in range(3):
                w = cp.tile([128, KS[l], DOUTS[l] + 8], bf, name=f"Waus{l}")
                for c in range(NCORES):
                    for kc in range(KS[l]):
                        nc.sync.dma_start(
                            w[:, kc, c * DCS[l]:(c + 1) * DCS[l]],
                            wout[c, :, WOFFS[l] + kc * DCS[l]:
                                 WOFFS[l] + (kc + 1) * DCS[l]])
                Waus.append(w)
            Wg1s = cp.tile([128, 8, 128], bf)
            for c in range(NCORES):
                nc.sync.dma_start(Wg1s[:, c, :],
                                  wout[c, :, WPOFF:WPOFF + 128])
            # layer state (x1 shipped fp8, converted to bf16 on device)
            x18 = cp.tile([128, NLOC], f8)
            nc.sync.dma_start(x18[:],
                              mega[:, C_X1T:C_X1T + NLOC // 2].bitcast(f8))
            xT1 = cp.tile([128, NLOC], bf)
            nc.vector.tensor_copy(xT1[:], x18[:])
            xT2 = cp.tile([128, 2 * NLOC], bf)
            xT3 = cp.tile([128, 4 * NLOC], bf)
            xTs = [xT1, xT2, xT3]
            h3T = cp.tile([128, 8 * NLOC], bf)
            ersb = [cp.tile([128, 4 * NTILES], bf, name=f"ersb{l}")
                    for l in range(3)]
            bsb = [cp.tile([128, DOUTS[l]], f32, name=f"bsb{l}")
                   for l in range(3)]
            # DRAM bounce buffers
            agin = [dp.tile([NLOC, DOUTS[l] + 4], bf, name=f"agin{l}")
                    for l in range(3)]
            agout = [dp.tile([NSTAR, DOUTS[l] + 4], bf, name=f"agout{l}",
                             addr_space="Shared") for l in range(3)]
            h3d = dp.tile([NLOC, D3], bf)

            with tc.tile_pool(name="psB", bufs=2, space="PSUM") as bp:
                for j in range(16):
                    pbT = bp.tile([1, 128], f32, tag="pbT")
                    nc.tensor.transpose(out=pbT[:],
                                        in_=smalls[:, 3 + j:4 + j],
                                        identity=idf32[:])
                    nc.vector.tensor_copy(bros[:, j * 128:(j + 1) * 128],
                                          pbT[:])

            for l in range(3):
                Dout, K = DOUTS[l], KS[l]
                ROW = Dout + 4
                O = Dout // H
                cks = _colchunks(Dout + 8)
                rcks = _colchunks(Dout)
                jer = next(j for j, (s, w) in enumerate(cks)
                           if s <= ROW < s + w)
                oer = ROW - cks[jer][0]

                # ---------------- P: feat/el/er projection (own slots)
                with tc.tile_pool(name=f"psP{l}", bufs=2, space="PSUM") as pp, \
                     tc.tile_pool(name=f"oP{l}", bufs=3) as op:
                    for t in range(NTILES):
                        pa = [pp.tile([128, w], f32, tag=f"pa{j}",
                                      name=f"pa{l}_{j}")
                              for j, (s, w) in enumerate(cks)]
                        for kc in range(K):
                            xt = xTs[l][:, kc * NLOC + t * 128:
                                        kc * NLOC + (t + 1) * 128]
                            for j, (s, w) in enumerate(cks):
                                nc.tensor.matmul(out=pa[j][:], lhsT=xt,
                                                 rhs=Waus[l][:, kc, s:s + w],
                                                 start=(kc == 0),
                                                 stop=(kc == K - 1))
                        ft = op.tile([128, ROW], bf, tag="ft")
                        for j, (s, w) in enumerate(cks):
                            lo, hi = s, min(s + w, ROW)
                            if hi > lo:
                                if j % 2 == 0:
                                    nc.scalar.copy(ft[:, lo:hi],
                                                   pa[j][:, :hi - lo])
                                else:
                                    nc.vector.tensor_copy(ft[:, lo:hi],
                                                          pa[j][:, :hi - lo])
                        nc.vector.tensor_copy(ersb[l][:, t * 4:(t + 1) * 4],
                                              pa[jer][:, oer:oer + 4])
                        nc.sync.dma_start(agin[l][t * 128:(t + 1) * 128, :],
                                          ft[:])
                    # bias -> [128, Dout] broadcast via ones-column matmul
                    bps = pp.tile([128, Dout], f32, tag="bias", bufs=1)
                    for j in range(Dout // 128):
                        jj = BOFF128[l] + j
                        nc.tensor.matmul(
                            out=bps[:, j * 128:(j + 1) * 128], lhsT=on1f[:],
                            rhs=bros[:, jj * 128:(jj + 1) * 128],
                            start=True, stop=True)
                    nc.vector.tensor_copy(bsb[l][:], bps[:])

                # ---------------- AllGather feat tables
                nc.gpsimd.collective_compute(
                    "AllGather", mybir.AluOpType.bypass,
                    ins=[agin[l].opt()], outs=[agout[l].opt()],
                    replica_groups=[list(range(NCORES))])

                # ---------------- L: gather / edge softmax / aggregate
                with tc.tile_pool(name=f"psS{l}", bufs=2, space="PSUM") as lp, \
                     tc.tile_pool(name=f"psR{l}", bufs=1, space="PSUM") as rp, \
                     tc.tile_pool(name=f"g{l}", bufs=3) as gp, \
                     tc.tile_pool(name=f"oh{l}", bufs=4) as ohp, \
                     tc.tile_pool(name=f"s{l}", bufs=2) as sp:
                    for t in range(NTILES):
                        rst = [rp.tile([128, w], f32, tag=f"rst{j}",
                                       name=f"rst{l}_{j}", bufs=1)
                               for j, (s, w) in enumerate(rcks)]
                        den = rp.tile([128, 4], f32, tag="den", bufs=1)
                        for c in range(CPT):
                            ch = t * CPT + c
                            gt = gp.tile([128, ROW], bf, tag="gt")
                            nc.gpsimd.indirect_dma_start(
                                out=gt[:], out_offset=None, in_=agout[l][:],
                                in_offset=bass.IndirectOffsetOnAxis(
                                    ap=idxs[:, ch:ch + 1], axis=0))
                            oh = ohp.tile([128, 128], bf, tag="oh")
                            nc.vector.scalar_tensor_tensor(
                                out=oh[:], in0=iotas[:], scalar=1.0,
                                in1=dposs[:, ch:ch + 1].to_broadcast(
                                    [128, 128]),
                                op0=AO.mult, op1=AO.is_equal)
                            pst = lp.tile([128, 128], bf, tag="pst")
                            nc.tensor.transpose(out=pst[:], in_=oh[:],
                                                identity=identb[:])
                            oht = ohp.tile([128, 128], bf, tag="oht")
                            nc.scalar.copy(oht[:], pst[:])
                            erp = lp.tile([128, 4], f32, tag="erp")
                            nc.tensor.matmul(
                                out=erp[:], lhsT=oht[:],
                                rhs=ersb[l][:, t * 4:(t + 1) * 4],
                                start=True, stop=True)
                            zz = sp.tile([128, 4], f32, tag="zz")
                            nc.vector.tensor_add(
                                zz[:], gt[:, Dout:Dout + 4], erp[:])
                            za = sp.tile([128, 4], f32, tag="za")
                            nc.vector.scalar_tensor_tensor(
                                out=za[:], in0=zz[:], scalar=NEG_SLOPE,
                                in1=zz[:], op0=AO.mult, op1=AO.max)
                            eef = sp.tile([128, 4], f32, tag="eef")
                            nc.scalar.activation(eef[:], za[:], FT.Exp)
                            eeb = sp.tile([128, 4], bf, tag="eeb")
                            nc.vector.tensor_copy(eeb[:], eef[:])
                            for h in range(H):
                                sl = gt[:, h * O:(h + 1) * O]
                                if h % 2 == 0:
                                    nc.vector.scalar_tensor_tensor(
                                        out=sl, in0=sl, scalar=1.0,
                                        in1=eeb[:, h:h + 1].to_broadcast(
                                            [128, O]),
                                        op0=AO.mult, op1=AO.mult)
                                else:
                                    nc.scalar.activation(
                                        sl, sl, FT.Copy,
                                        scale=eef[:, h:h + 1])
                            nc.tensor.matmul(out=den[:], lhsT=oh[:],
                                             rhs=eeb[:],
                                             start=(c == 0),
                                             stop=(c == CPT - 1))
                            for j, (s, w) in enumerate(rcks):
                                nc.tensor.matmul(
                                    out=rst[j][:], lhsT=oh[:],
                                    rhs=gt[:, s:s + w],
                                    start=(c == 0),
                                    stop=(c == CPT - 1))
                        dcl = sp.tile([128, 4], f32, tag="dcl")
                        nc.vector.tensor_scalar_max(dcl[:], den[:], 1e-9)
                        rec = sp.tile([128, 4], f32, tag="rec")
                        nc.vector.reciprocal(rec[:], dcl[:])
                        y = sp.tile([128, Dout], f32, tag="y")
                        for h in range(H):
                            j = (h * O) // 512
                            s0 = (h * O) % 512
                            nc.vector.scalar_tensor_tensor(
                                out=y[:, h * O:(h + 1) * O],
                                in0=rst[j][:, s0:s0 + O],
                                scalar=rec[:, h:h + 1],
                                in1=bsb[l][:, h * O:(h + 1) * O],
                                op0=AO.mult, op1=AO.add)
                        mn = sp.tile([128, Dout], f32, tag="mn")
                        nc.vector.tensor_scalar_min(mn[:], y[:], 0.0)
                        ex = sp.tile([128, Dout], f32, tag="ex")
                        nc.scalar.activation(ex[:], mn[:], FT.Exp)
                        y2 = sp.tile([128, Dout], f32, tag="y2")
                        nc.vector.scalar_tensor_tensor(
                            out=y2[:], in0=y[:], scalar=0.0, in1=ex[:],
                            op0=AO.max, op1=AO.add)
                        xo = sp.tile([128, Dout], bf, tag="xo")
                        nc.vector.tensor_scalar_add(xo[:], y2[:], -1.0)
                        # transpose into next layer's lhsT layout
                        dstT = xTs[l + 1] if l < 2 else h3T
                        if l == 2:
                            nc.sync.dma_start(h3d[t * 128:(t + 1) * 128, :],
                                              xo[:])
                        for kc in range(Dout // 128):
                            psx = lp.tile([128, 128], bf, tag="pst",
                                          name="psx")
                            nc.tensor.transpose(out=psx[:],
                                                in_=xo[:, kc * 128:
                                                       (kc + 1) * 128],
                                                identity=identb[:])
                            tgt = dstT[:, kc * NLOC + t * 128:
                                       kc * NLOC + (t + 1) * 128]
                            if kc % 2 == 0:
                                nc.scalar.copy(tgt, psx[:])
                            else:
                                nc.vector.tensor_copy(tgt, psx[:])

            if debug:
                nc.sync.dma_start(dbgA[:], agout[0][:])
                nc.sync.dma_start(dbgB[:], xT2[:])
                nc.sync.dma_start(dbgC[:], ersb[0][:])

            # ---------------- POOL: gate MLP, per-graph softmax, latent heads
            with tc.tile_pool(name="psQ", bufs=2, space="PSUM") as qp, \
                 tc.tile_pool(name="sq", bufs=3) as sq:
                on32b = sq.tile([1, 32], bf, tag="on32", bufs=1)
                nc.vector.memset(on32b[:], 1.0)
                # graph one-hots (per tile) from gid columns
                GOHs = sq.tile([128, NTILES * GPAD], bf, tag="goh", bufs=1)
                for t in range(NTILES):
                    nc.vector.scalar_tensor_tensor(
                        out=GOHs[:, t * GPAD:(t + 1) * GPAD],
                        in0=iotas[:, :GPAD], scalar=1.0,
                        in1=gids[:, t:t + 1].to_broadcast([128, GPAD]),
                        op0=AO.mult, op1=AO.is_equal)
                relu1 = sq.tile([128, NLOC], bf, tag="relu1", bufs=1)
                nwin = (NLOC + 511) // 512
                for w in range(nwin):
                    s = w * 512
                    ww = min(512, NLOC - s)
                    ps = qp.tile([128, 512], f32, tag="g1")
                    for kc in range(8):
                        nc.tensor.matmul(out=ps[:, :ww], lhsT=Wg1s[:, kc, :],
                                         rhs=h3T[:, kc * NLOC + s:
                                                 kc * NLOC + s + ww],
                                         start=(kc == 0), stop=(kc == 7))
                    nc.scalar.activation(relu1[:, s:s + ww], ps[:, :ww],
                                         FT.Relu, bias=smalls[:, 0:1])
                gps = qp.tile([128, 32], f32, tag="g2", bufs=1)
                for t in range(NTILES):
                    nc.tensor.matmul(out=gps[:, t:t + 1],
                                     lhsT=relu1[:, t * 128:(t + 1) * 128],
                                     rhs=Wg2c[:], start=True, stop=True)
                eg = sq.tile([128, NTILES], bf, tag="eg", bufs=1)
                nc.scalar.activation(eg[:], gps[:, :NTILES], FT.Exp,
                                     bias=smalls[:, 2:3])
                gd = qp.tile([GPAD, 1], f32, tag="gd", bufs=1)
                goha = sq.tile([128, NTILES * GPAD], bf, tag="goha", bufs=1)
                for t in range(NTILES):
                    nc.tensor.matmul(out=gd[:],
                                     lhsT=GOHs[:, t * GPAD:(t + 1) * GPAD],
                                     rhs=eg[:, t:t + 1],
                                     start=(t == 0), stop=(t == NTILES - 1))
                    nc.vector.tensor_mul(
                        goha[:, t * GPAD:(t + 1) * GPAD],
                        GOHs[:, t * GPAD:(t + 1) * GPAD],
                        eg[:, t:t + 1].to_broadcast([128, GPAD]))
                geps = [qp.tile([GPAD, 512], f32, tag=f"ge{j}",
                                name=f"geps{j}", bufs=1) for j in range(2)]
                for t in range(NTILES):
                    h3t = sq.tile([128, D3], bf, tag="h3t")
                    nc.sync.dma_start(h3t[:], h3d[t * 128:(t + 1) * 128, :])
                    for j in range(2):
                        nc.tensor.matmul(out=geps[j][:],
                                         lhsT=goha[:, t * GPAD:(t + 1) * GPAD],
                                         rhs=h3t[:, j * 512:(j + 1) * 512],
                                         start=(t == 0),
                                         stop=(t == NTILES - 1))
                gdc = sq.tile([GPAD, 1], f32, tag="gdc", bufs=1)
                nc.vector.tensor_scalar_max(gdc[:], gd[:], 1e-9)
                grc = sq.tile([GPAD, 1], f32, tag="grc", bufs=1)
                nc.vector.reciprocal(grc[:], gdc[:])
                zge = sq.tile([GPAD, D3], f32, tag="zge", bufs=1)
                nc.vector.memset(zge[:], 0.0)
                ge = sq.tile([GPAD, D3], f32, tag="ge", bufs=1)
                for j in range(2):
                    nc.vector.scalar_tensor_tensor(
                        out=ge[:, j * 512:(j + 1) * 512], in0=geps[j][:],
                        scalar=grc[:, 0:1], in1=zge[:, j * 512:(j + 1) * 512],
                        op0=AO.mult, op1=AO.add)
                geT = sq.tile([128, 8 * GPAD], bf, tag="geT", bufs=1)
                for kc in range(8):
                    psq = qp.tile([128, GPAD], f32, tag="pstq", bufs=1)
                    nc.tensor.transpose(out=psq[:],
                                        in_=ge[:, kc * 128:(kc + 1) * 128],
                                        identity=idf32[:GPAD, :GPAD])
                    nc.vector.tensor_copy(geT[:, kc * GPAD:(kc + 1) * GPAD],
                                          psq[:])
                for oi, (wi, brow) in enumerate([(1, 14), (2, 15)]):
                    Ws = sq.tile([128, 8, 128], bf, tag="wmlv", name=f"Wl{oi}")
                    for c in range(NCORES):
                        nc.sync.dma_start(
                            Ws[:, c, :],
                            wout[c, :, WPOFF + wi * 128:
                                 WPOFF + (wi + 1) * 128])
                    bmub = sq.tile([1, 128], bf, tag="bmub", bufs=2,
                                   name=f"bmub{oi}")
                    nc.vector.tensor_copy(bmub[:],
                                          bros[:, brow * 128:
                                               (brow + 1) * 128])
                    mps = qp.tile([GPAD, 128], f32, tag="mps", bufs=1)
                    for kc in range(8):
                        nc.tensor.matmul(
                            out=mps[:],
                            lhsT=geT[:, kc * GPAD:(kc + 1) * GPAD],
                            rhs=Ws[:, kc, :],
                            start=(kc == 0), stop=False)
                    nc.tensor.matmul(out=mps[:], lhsT=on32b[:],
                                     rhs=bmub[:], start=False, stop=True)
                    mo = sq.tile([GPAD, 128], bf, tag="mo")
                    nc.vector.tensor_copy(mo[:], mps[:])
                    nc.sync.dma_start(mulv[:, oi * 128:(oi + 1) * 128], mo[:])
    nc.compile()
    return nc


_BUILD_CACHE = {}


def _get(key, fn):
    if key not in _BUILD_CACHE:
        _BUILD_CACHE[key] = fn()
    return _BUILD_CACHE[key]


_RUNNER_CACHE = {}


def _make_runner(nc):
    """Like bass2jax.run_bass_via_pjrt, but the jitted shard_map callable is
    built once so repeat calls skip tracing/XLA-compile and only pay input
    transfer + execution."""
    import jax
    import numpy as _np
    from jax.sharding import Mesh, PartitionSpec
    from jax.experimental.shard_map import shard_map
    from concourse import mybir
    from concourse.bass2jax import (_bass_exec_p, install_neuronx_cc_hook,
                                    partition_id_tensor)

    install_neuronx_cc_hook()
    partition_name = (nc.partition_id_tensor.name
                      if nc.partition_id_tensor else None)
    in_names, out_names, out_avals, zero_outs = [], [], [], []
    for alloc in nc.m.functions[0].allocations:
        if not isinstance(alloc, mybir.MemoryLocationSet):
            continue
        name = alloc.memorylocations[0].name
        if alloc.kind == "ExternalInput":
            if name != partition_name:
                in_names.append(name)
        elif alloc.kind == "ExternalOutput":
            out_names.append(name)
            shape = tuple(alloc.tensor_shape)
            dtype = mybir.dt.np(alloc.dtype)
            out_avals.append(jax.core.ShapedArray(shape, dtype))
            zero_outs.append(_np.zeros(shape, dtype))
    n_params = len(in_names)
    n_outs = len(out_avals)
    all_names = list(in_names) + list(out_names)
    if partition_name is not None:
        all_names.append(partition_name)

    def _body(*args):
        operands = list(args)
        if partition_name is not None:
            operands.append(partition_id_tensor())
        return tuple(_bass_exec_p.bind(
            *operands, out_avals=tuple(out_avals), in_names=tuple(all_names),
            out_names=tuple(out_names), lowering_input_output_aliases=(),
            sim_require_finite=True, sim_require_nnan=True, nc=nc))

    devices = jax.devices()[:NCORES]
    mesh = Mesh(np.asarray(devices), ("core",))
    donate = tuple(range(n_params, n_params + n_outs))
    sharded = jax.jit(
        shard_map(_body, mesh=mesh,
                  in_specs=(PartitionSpec("core"),) * (n_params + n_outs),
                  out_specs=(PartitionSpec("core"),) * n_outs,
                  check_rep=False),
        donate_argnums=donate, keep_unused=True)

    def run(in_maps):
        concat_in = [np.concatenate([np.asarray(m[name]) for m in in_maps], 0)
                     for name in in_names]
        concat_zeros = [np.zeros((NCORES * z.shape[0], *z.shape[1:]), z.dtype)
                        for z in zero_outs]
        out_arrs = sharded(*concat_in, *concat_zeros)
        return [{name: np.asarray(out_arrs[i]).reshape(
                    NCORES, *out_avals[i].shape)[c]
                 for i, name in enumerate(out_names)}
                for c in range(NCORES)]

    return run


def _run(nc, in_maps):
    key = id(nc)
    if key not in _RUNNER_CACHE:
        _RUNNER_CACHE[key] = _make_runner(nc)
    return _RUNNER_CACHE[key](in_maps)


# ------------------------------------------------------------------ main entry
def kernel(node_feat, src, dst, graph_ids,
           W1, al1, ar1, b1, W2, al2, ar2, b2, W3, al3, ar3, b3,
           Wg1, bg1, Wg2, bg2, Wmu, bmu, Wlv, blv):
    prep = host_prep(node_feat, src, dst, graph_ids)
    Waus = [wstack(fold_weights(np.asarray(W, np.float32),
                                np.asarray(al, np.float32),
                                np.asarray(ar, np.float32))).astype(BF16)
            for W, al, ar in [(W1, al1, ar1), (W2, al2, ar2), (W3, al3, ar3)]]
    wpool = [wstack(np.asarray(W, np.float32)).astype(BF16)
             for W in (Wg1, Wmu, Wlv)]
    brows = np.concatenate([np.asarray(b, np.float32)
                            for b in (b1, b2, b3)])
    smalls = np.zeros((128, NSM), np.float32)
    smalls[:, 0] = np.asarray(bg1, np.float32)
    smalls[:, 1] = np.asarray(Wg2, np.float32).reshape(-1)
    smalls[:, 2] = np.asarray(bg2, np.float32).reshape(-1)[0]
    smalls[:, 3:17] = brows.reshape(14, 128).T
    smalls[:, 17] = np.asarray(bmu, np.float32)
    smalls[:, 18] = np.asarray(blv, np.float32)

    ncf = _get("fused", build_fused)
    in_maps = []
    for c in range(NCORES):
        mega = np.empty((128, MCOLS), BF16)
        mega[:, C_X1T:C_X1T + NLOC // 2] = prep["x1T"][c].view(BF16)
        mega[:, C_DPOS:C_DPOS + NCH // 2] = prep["dpos"][c].view(BF16)
        mega[:, C_GID:C_GID + NTILES] = prep["gid"][c]
        mega[:, C_IDX:C_IDX + NCH] = prep["idx"][c].view(BF16)
        for l in range(3):
            sh = np.ascontiguousarray(
                Waus[l][:, :, c * DCS[l]:(c + 1) * DCS[l]]).reshape(128, -1)
            mega[:, C_W + WOFFS[l]:C_W + WOFFS[l] + KS[l] * DCS[l]] = sh
        for wi in range(3):
            mega[:, C_W + WPOFF + wi * 128:
                 C_W + WPOFF + (wi + 1) * 128] = wpool[wi][:, c, :]
        mega[:, C_SM:C_SM + 2 * NSM] = smalls.view(BF16)
        in_maps.append(dict(mega=mega))
    out = _run(ncf, in_maps)
    mu = np.concatenate([out[c]["mulv"][:GPC, :128] for c in range(NCORES)], 0)
    lv = np.concatenate([out[c]["mulv"][:GPC, 128:] for c in range(NCORES)], 0)
    return np.asarray(mu, np.float32), np.asarray(lv, np.float32)
